# revision 1
# baseline (speedup 1.0000x reference)
"""Trainium2 Bass kernel for BEiT attention block (nn_Beit_9560597201107).

Data-parallel over batch: 64 batches -> 8 NeuronCores x 8 batches each.
Fully transposed dataflow (channels on partitions) so the softmax'd
attention matrix is never transposed on-chip:

  xT = x.T (PE transpose)                          [768, 197]
  qkT[c, n] = sum_k WT[k, c] xT[k, n] + bias       [1536, 197]  (q pre-scaled)
  v[m, d]   = sum_k xT[k, m] WT_v[k, d] + bias     [197, 768]   (natural)
  scT[m, n] = sum_d kT[d, m] qT[d, n]              per head
  eT = exp(scT) * exp_rel_T                        (rel bias via exp-mult)
  sums[h, n] = sum_m eT[m, n]   (ones-column matmul)
  po[d, n]  = sum_m v[m, d] eT[m, n]               (unnormalized outT)
  cT = po * broadcast(1/sums)   (PE ones-outer-product broadcast)
  y[n, o] = sum_c cT[c, n] projWT[c, o] + bias

All matmuls run in float32r (full-rate fp32, ~1e-4 relative rounding);
free dims padded to 256 to stay at 1 cycle/row.
"""

import os
import numpy as np

import concourse.bass as bass
import concourse.bacc as bacc
import concourse.mybir as mybir
import concourse.tile as tile
from concourse.bass_utils import run_bass_kernel_spmd
from concourse.bass_interp import get_hw_module
B, N, DIM, HEADS, NBS = 64, 197, 768, 12, 10
HEAD_DIM = DIM // HEADS
SCALE = HEAD_DIM ** -0.5
NCORES = 8
BPC = B // NCORES          # batches per core
KT = DIM // 128            # 6 contraction tiles
NPAD = 256                 # padded token free-dim (fp32r needs >=256 for full rate)
TOK_TILES = [(0, 128), (128, 69)]  # (offset, size) over the 197 tokens
# Scores head-pairs grouped by parity: both heads of a pair live at the same
# 64-partition half of qkT, so their back-to-back matmuls into one PSUM bank
# use the same PE row group (mixed row groups on one bank crash fp32r).
PAIRS = [(0, 2), (4, 6), (8, 10), (1, 3), (5, 7), (9, 11)]
PAIR_PERM = [h for p in PAIRS for h in p]

F32 = mybir.dt.float32
F32R = mybir.dt.float32r

_CACHE = {}


def _build_module():
    nc = bacc.Bacc("TRN2", target_bir_lowering=False, debug=False)

    # host-transposed, zero-padded x: xt8[b, k, p, n] = x[b, n, 128k+p]
    xt8_d = nc.dram_tensor("xt8", [BPC, KT, 128, NPAD], F32, kind="ExternalInput")
    wt_d = nc.dram_tensor("wt", [KT, 128, 3 * DIM], F32, kind="ExternalInput")
    pwt_d = nc.dram_tensor("pwt", [KT, 128, DIM], F32, kind="ExternalInput")
    qbc_d = nc.dram_tensor("qbc", [128, BPC, KT], F32, kind="ExternalInput")
    vpb_d = nc.dram_tensor("vpb8", [BPC, DIM], F32, kind="ExternalInput")
    relt_d = nc.dram_tensor("relt", [6, 2, 128, 2 * NPAD], F32, kind="ExternalInput")
    ones_d = nc.dram_tensor("ones1", [1, NPAD], F32, kind="ExternalInput")
    oh3_d = nc.dram_tensor("oh3", [128, 3, 65], F32, kind="ExternalInput")
    allones_d = nc.dram_tensor("allones", [128, 64], F32, kind="ExternalInput")
    y8_d = nc.dram_tensor("y8", [BPC, N, DIM], F32, kind="ExternalOutput")

    with tile.TileContext(nc) as tc:
        with (
            tc.tile_pool(name="const", bufs=1) as constp,
            tc.tile_pool(name="sb_xT", bufs=1) as sb_xT,
            tc.tile_pool(name="sb_qkT", bufs=1) as sb_qkT,
            tc.tile_pool(name="sb_v", bufs=2) as sb_v,
            tc.tile_pool(name="sb_exp", bufs=2) as sb_exp,
            tc.tile_pool(name="sb_po", bufs=8) as sb_po,
            tc.tile_pool(name="sb_ctmp", bufs=2) as sb_ctmp,
            tc.tile_pool(name="sb_pbs", bufs=2) as sb_pbs,
            tc.tile_pool(name="sb_rec", bufs=2) as sb_rec,
            tc.tile_pool(name="sb_cT", bufs=2) as sb_cT,
            tc.tile_pool(name="sb_out", bufs=2) as sb_out,
            tc.tile_pool(name="sb_vpb", bufs=2) as sb_vpb,
            tc.tile_pool(name="ps", bufs=6, space="PSUM") as ps,
            tc.tile_pool(name="ps_sums", bufs=2, space="PSUM") as ps_sums,
        ):
            # ---- persistent data (loaded once) ----
            wt_sb = constp.tile([128, KT, 3 * DIM], F32R)
            nc.gpsimd.dma_start(out=wt_sb[:], in_=wt_d.ap().transpose([1, 0, 2]))
            pwt_sb = constp.tile([128, KT, DIM], F32R)
            nc.gpsimd.dma_start(out=pwt_sb[:], in_=pwt_d.ap().transpose([1, 0, 2]))
            relt_sb = constp.tile([128, 6, 2, 2 * NPAD], F32R)
            nc.gpsimd.dma_start(out=relt_sb[:], in_=relt_d.ap().transpose([2, 0, 1, 3]))
            qbc_sb = constp.tile([128, BPC, KT], F32)
            nc.sync.dma_start(out=qbc_sb[:], in_=qbc_d.ap())

            ones_sb = constp.tile([1, NPAD], F32R)
            nc.gpsimd.dma_start(out=ones_sb[:], in_=ones_d.ap())
            oh3_sb = constp.tile([128, 3, 65], F32R)
            nc.gpsimd.dma_start(out=oh3_sb[:], in_=oh3_d.ap())
            allones_sb = constp.tile([128, 64], F32R)
            nc.gpsimd.dma_start(out=allones_sb[:], in_=allones_d.ap())

            def kT(qkT_sb, h, hb, off, mt):
                base = (h % 2) * 64
                return qkT_sb[base:base + 64, 6 + h // 2,
                              hb * NPAD + off:hb * NPAD + off + mt]

            def qT(qkT_sb, h, hb):
                base = (h % 2) * 64
                return qkT_sb[base:base + 64, h // 2, hb * NPAD:(hb + 1) * NPAD]

            prev_proj = [None]
            proj_state = {}

            def emit_proj_chunk(step, drain=False):
                if prev_proj[0] is None:
                    return
                pb_, cT_, vpb_ = prev_proj[0]
                if step == 0 and not drain:
                    proj_state.clear()
                chunks = [(0, 0), (0, 1), (0, 2), (1, 0), (1, 1), (1, 2)]
                todo = chunks if drain else [chunks[step]]
                for (t, jc) in todo:
                    off, mt = TOK_TILES[t]
                    if jc == 0:
                        proj_state[t] = (
                            ps.tile([128, 512], F32, tag="ps", name=f"pr_{pb_}_{t}"),
                            ps.tile([128, NPAD], F32, tag="ps", name=f"pr2_{pb_}_{t}"),
                        )
                    pr, pr2 = proj_state[t]
                    for j in (2 * jc, 2 * jc + 1):
                        nc.tensor.matmul(
                            pr[0:mt, :], cT_[:, j, off:off + mt], pwt_sb[:, j, 0:512],
                            start=(j == 0), stop=False,
                        )
                        nc.tensor.matmul(
                            pr2[0:mt, :], cT_[:, j, off:off + mt], pwt_sb[:, j, 512:768],
                            start=(j == 0), stop=False,
                        )
                    if jc == 2:
                        nc.tensor.matmul(
                            pr[0:mt, :], ones_sb[0:1, 0:mt], vpb_[0:1, 0:512],
                            start=False, stop=True,
                        )
                        nc.tensor.matmul(
                            pr2[0:mt, :], ones_sb[0:1, 0:mt], vpb_[0:1, 512:768],
                            start=False, stop=True,
                        )
                        out_sb = sb_out.tile([128, DIM], F32, tag="out",
                                             name=f"out_{pb_}_{t}")
                        nc.scalar.copy(out_sb[0:mt, 0:512], pr[0:mt, :])
                        nc.vector.tensor_copy(out_sb[0:mt, 512:768], pr2[0:mt, :])
                        nc.sync.dma_start(out=y8_d.ap()[pb_, off:off + mt, :],
                                          in_=out_sb[0:mt, :])
                if drain:
                    prev_proj[0] = None

            for g in range(BPC // 2):
                # ---- load host-transposed x for the batch pair ----
                xT_sb = sb_xT.tile([128, KT, 2 * NPAD], F32R, tag="xT", name=f"xT_{g}")
                for hb in range(2):
                    nc.gpsimd.dma_start(
                        out=xT_sb[:, :, hb * NPAD:(hb + 1) * NPAD],
                        in_=xt8_d.ap()[2 * g + hb].transpose([1, 0, 2]),
                    )

                # ---- qkT for both batches (one weight load per block) ----
                qkT_sb = sb_qkT.tile([128, 12, 2 * NPAD], F32R, tag="qkT", name=f"qkT_{g}")
                for ct in range(12):
                    qp = ps.tile([128, 512], F32, tag="ps", name=f"qp_{g}_{ct}")
                    for k in range(KT):
                        nc.tensor.matmul(
                            qp[:],
                            wt_sb[:, k, ct * 128:(ct + 1) * 128],
                            xT_sb[:, k, :],
                            start=(k == 0),
                            stop=(k == KT - 1),
                        )
                    if ct < 6:
                        for hb in range(2):
                            qbias = qbc_sb[:, 2 * g + hb, ct:ct + 1]
                            dst = qkT_sb[:, ct, hb * NPAD:(hb + 1) * NPAD]
                            srcp = qp[:, hb * NPAD:(hb + 1) * NPAD]
                            if ct % 2 == 0:
                                nc.vector.tensor_scalar_add(dst, srcp, qbias)
                            else:
                                nc.scalar.activation(
                                    dst, srcp,
                                    mybir.ActivationFunctionType.Identity, bias=qbias,
                                )
                    else:
                        if ct % 2 == 0:
                            nc.vector.tensor_copy(qkT_sb[:, ct, :], qp[:])
                        else:
                            nc.scalar.copy(qkT_sb[:, ct, :], qp[:])

                for hb in range(2):
                    b = 2 * g + hb

                    vpb_t = sb_vpb.tile([1, DIM], F32R, tag="vpb", name=f"vpb_{b}")
                    nc.gpsimd.dma_start(out=vpb_t[:], in_=vpb_d.ap()[b].unsqueeze(0))

                    # ---- v (natural layout) ----
                    v_sb = sb_v.tile([128, 2, HEADS, HEAD_DIM], F32R, tag="v",
                                     name=f"v_{b}")
                    for t, (off, mt) in enumerate(TOK_TILES):
                        vp = ps.tile([128, 512], F32, tag="ps", name=f"vp_{b}_{t}")
                        vp2 = ps.tile([128, NPAD], F32, tag="ps", name=f"vp2_{b}_{t}")
                        for k in range(KT):
                            xsl = xT_sb[:, k, hb * NPAD + off:hb * NPAD + off + mt]
                            nc.tensor.matmul(
                                vp[0:mt, :], xsl, wt_sb[:, k, 1536:2048],
                                start=(k == 0), stop=(k == KT - 1),
                            )
                            nc.tensor.matmul(
                                vp2[0:mt, :], xsl, wt_sb[:, k, 2048:2304],
                                start=(k == 0), stop=(k == KT - 1),
                            )
                        # v_sb head axis is in PAIR_PERM order: even head h ->
                        # slot h//2, odd head h -> slot 6 + h//2
                        nc.vector.tensor_copy(
                            v_sb[0:mt, t, :, :].rearrange(
                                "p (par a) d -> p a par d", par=2)[:, 0:4, :, :],
                            vp[0:mt, :].rearrange("p (a par d) -> p a par d",
                                                  par=2, d=HEAD_DIM),
                        )
                        nc.scalar.copy(
                            v_sb[0:mt, t, :, :].rearrange(
                                "p (par a) d -> p a par d", par=2)[:, 4:6, :, :],
                            vp2[0:mt, :].rearrange("p (a par d) -> p a par d",
                                                   par=2, d=HEAD_DIM),
                        )

                    # ---- attention, with prev-batch proj interleaved ----
                    sums_pA = ps_sums.tile([65, 512], F32, tag="sums", name=f"sumsA_{b}")
                    sums_pB = ps_sums.tile([65, 512], F32, tag="sums", name=f"sumsB_{b}")
                    po_sb_by_pair = {}
                    for step, sp in enumerate([0, 3, 1, 4, 2, 5]):
                        h0, h1 = PAIRS[sp]
                        expT = sb_exp.tile([128, 2, 2 * NPAD], F32R, tag="expT",
                                           name=f"expT_{b}_{sp}")
                        sums_px = sums_pA if sp < 3 else sums_pB
                        j3 = sp % 3
                        scs = []
                        for t, (off, mt) in enumerate(TOK_TILES):
                            sc = ps.tile([128, 512], F32, tag="ps", name=f"sc_{b}_{sp}_{t}")
                            nc.tensor.matmul(
                                sc[0:mt, 0:NPAD], kT(qkT_sb, h0, hb, off, mt),
                                qT(qkT_sb, h0, hb), start=True, stop=True,
                            )
                            nc.tensor.matmul(
                                sc[0:mt, NPAD:512], kT(qkT_sb, h1, hb, off, mt),
                                qT(qkT_sb, h1, hb), start=True, stop=True,
                            )
                            scs.append(sc)
                        for t, (off, mt) in enumerate(TOK_TILES):
                            nc.scalar.activation(
                                expT[0:mt, t, :], scs[t][0:mt, :],
                                mybir.ActivationFunctionType.Exp,
                            )
                            nc.vector.tensor_mul(
                                expT[0:mt, t, :], expT[0:mt, t, :],
                                relt_sb[0:mt, sp, t, :],
                            )
                        # long warm proj matmuls of the previous batch fill the
                        # exp/mult wait and keep the PE clock-gate open
                        emit_proj_chunk(step)
                        po = ps.tile([128, 512], F32, tag="ps", name=f"po_{b}_{sp}")
                        for t, (off, mt) in enumerate(TOK_TILES):
                            nc.tensor.matmul(
                                po[:, :], v_sb[0:mt, t, 2 * sp:2 * sp + 2, :],
                                expT[0:mt, t, :], start=(t == 0), stop=(t == 1),
                            )
                            nc.tensor.matmul(
                                sums_px[0:65, :], oh3_sb[0:mt, j3, :], expT[0:mt, t, :],
                                start=(sp in (0, 3) and t == 0),
                                stop=(sp in (2, 5) and t == 1),
                                skip_group_check=True,
                            )
                        po_sb = sb_po.tile([128, NPAD], F32, tag="po",
                                           name=f"po_sb_{b}_{sp}")
                        nc.scalar.copy(po_sb[0:64, :], po[0:64, 0:NPAD])
                        nc.vector.tensor_copy(po_sb[64:128, :], po[64:128, NPAD:512])
                        po_sb_by_pair[sp] = po_sb

                    rec_fA = sb_rec.tile([65, 512], F32, tag="recf", name=f"recfA_{b}")
                    rec_fB = sb_rec.tile([65, 512], F32, tag="recf", name=f"recfB_{b}")
                    nc.vector.reciprocal_approx_fast(out=rec_fA[0:65, :],
                                                     in_=sums_pA[0:65, :])
                    nc.vector.reciprocal_approx_fast(out=rec_fB[0:65, :],
                                                     in_=sums_pB[0:65, :])
                    rec_sbA = sb_rec.tile([65, 512], F32R, tag="rec", name=f"recA_{b}")
                    rec_sbB = sb_rec.tile([65, 512], F32R, tag="rec", name=f"recB_{b}")
                    nc.scalar.copy(rec_sbA[0:65, :], rec_fA[0:65, :])
                    nc.scalar.copy(rec_sbB[0:65, :], rec_fB[0:65, :])

                    cT_sb = sb_cT.tile([128, KT, NPAD], F32R, tag="cT", name=f"cT_{b}")
                    for sp in range(6):
                        rec_x = rec_sbA if sp < 3 else rec_sbB
                        r0 = 32 * (sp % 3)
                        pb2 = ps.tile([64, 512], F32, tag="ps", name=f"pb_{b}_{sp}")
                        nc.tensor.matmul(
                            pb2[0:64, :], allones_sb[r0:r0 + 1, 0:64],
                            rec_x[r0:r0 + 1, 0:512],
                            start=True, stop=True,
                        )
                        pblo = sb_ctmp.tile([64, NPAD], F32, tag="ctmp",
                                            name=f"pblo_{b}_{sp}")
                        nc.scalar.copy(pblo[:], pb2[0:64, NPAD:512])
                        pbhi = sb_pbs.tile([128, NPAD], F32, tag="pbs",
                                           name=f"pbhi_{b}_{sp}")
                        nc.sync.dma_start(out=pbhi[64:128, :], in_=pblo[:])
                        po_sb = po_sb_by_pair[sp]
                        nc.vector.tensor_mul(cT_sb[0:64, sp, :], po_sb[0:64, :],
                                             pb2[0:64, 0:NPAD])
                        nc.vector.tensor_mul(cT_sb[64:128, sp, :], po_sb[64:128, :],
                                             pbhi[64:128, :])
                    prev_proj[0] = (b, cT_sb, vpb_t)

            # drain the last batch's projection
            emit_proj_chunk(0, drain=True)

    nc.compile()
    nc.m = get_hw_module(nc.m)
    return nc


def _host_prep(x, qkv_weight, q_bias, v_bias, rel_table, proj_weight, proj_bias,
               b_idx, rel_index):
    x = np.asarray(x, dtype=np.float32)
    # xt8[b, k, p, n] = x[b, n, 128k+p], zero-padded to NPAD tokens
    xt = np.zeros((B, KT, 128, NPAD), dtype=np.float32)
    xt[:, :, :, 0:N] = x.transpose(0, 2, 1).reshape(B, KT, 128, N)
    W = np.asarray(qkv_weight, dtype=np.float32).copy()
    W[:DIM] *= np.float32(SCALE)
    wt = np.ascontiguousarray(W.T.reshape(KT, 128, 3 * DIM))
    pwtT = np.asarray(proj_weight, dtype=np.float32).T  # [c', o]
    pwtT = pwtT.reshape(HEADS, HEAD_DIM, DIM)[PAIR_PERM].reshape(DIM, DIM)
    pwt = np.ascontiguousarray(pwtT.reshape(KT, 128, DIM))

    bi = np.asarray(b_idx).astype(np.int64)
    qb_all = (np.asarray(q_bias, dtype=np.float32)[bi] * np.float32(SCALE))
    vb_all = np.asarray(v_bias, dtype=np.float32)[bi]
    # softmax rows sum to 1, so attn @ (1 x vb) == 1 x vb; push the v bias
    # through the projection into the proj bias
    pb_all = (np.asarray(proj_bias, dtype=np.float32)[bi]
              + vb_all @ np.asarray(proj_weight, dtype=np.float32).T)

    ridx = np.asarray(rel_index).astype(np.int64)
    rel = np.asarray(rel_table, dtype=np.float32)[ridx.reshape(-1)]
    rel = rel.reshape(N, N, HEADS)  # [n, m, h]
    relth = np.zeros((HEADS, 2, 128, NPAD), dtype=np.float32)
    for t, (off, mt) in enumerate(TOK_TILES):
        # relth[h, t, p, n] = exp(rel[n, off+p, h])
        relth[:, t, 0:mt, 0:N] = np.exp(rel[:, off:off + mt, :].transpose(2, 1, 0))
    # pair-merged: relt[sp, t, p, i*NPAD+n] = relth[PAIRS[sp][i], t, p, n]
    relt = np.ascontiguousarray(
        relth[PAIR_PERM].reshape(6, 2, 2, 128, NPAD)
        .transpose(0, 2, 3, 1, 4).reshape(6, 2, 128, 2 * NPAD))

    ones1 = np.zeros((1, NPAD), dtype=np.float32)
    ones1[0, 0:N] = 1.0
    oh3 = np.zeros((128, 3, 65), dtype=np.float32)
    for j in range(3):
        oh3[:, j, 32 * j] = 1.0
    allones = np.ones((128, 64), dtype=np.float32)

    in_maps = []
    for c in range(NCORES):
        sl = slice(c * BPC, (c + 1) * BPC)
        qbc = np.ascontiguousarray(
            qb_all[sl].reshape(BPC, KT, 128).transpose(2, 0, 1))
        vpb = np.ascontiguousarray(pb_all[sl])
        in_maps.append({
            "xt8": np.ascontiguousarray(xt[sl]),
            "wt": wt,
            "pwt": pwt,
            "qbc": qbc,
            "vpb8": vpb,
            "relt": relt,
            "ones1": ones1,
            "oh3": oh3,
            "allones": allones,
        })
    return in_maps


def _install_ntff_hook():
    """Provide antenv.axon_hooks (absent from this image) so bass_utils can
    capture NTFF profiles through libaxon_pjrt.so, and keep artifacts local."""
    if _CACHE.get("hook_installed"):
        return
    import sys
    import types
    import ctypes
    import contextlib

    so_path = "/opt/axon/libaxon_pjrt.so"
    lib = ctypes.CDLL(so_path)
    lib.axon_start_nrt_profile.argtypes = [
        ctypes.POINTER(ctypes.c_int64),
        ctypes.c_size_t,
    ]
    lib.axon_start_nrt_profile.restype = ctypes.c_int64
    lib.axon_stop_nrt_profile.argtypes = [ctypes.c_char_p]
    lib.axon_stop_nrt_profile.restype = ctypes.c_int64

    @contextlib.contextmanager
    def _hook(output_dir, device_ids):
        import jax

        jax.devices()
        if device_ids:
            ids = (ctypes.c_int64 * len(device_ids))(*device_ids)
            rc = lib.axon_start_nrt_profile(ids, len(device_ids))
        else:
            rc = lib.axon_start_nrt_profile(None, 0)
        if rc != 0:
            raise RuntimeError(f"axon_start_nrt_profile rc={rc}")
        try:
            yield
        finally:
            n = lib.axon_stop_nrt_profile(str(output_dir).encode())
            print(f"ntff profile: {n} file(s) written to {output_dir}")

    mod = types.ModuleType("antenv.axon_hooks")
    mod.get_axon_ntff_profile_hook = lambda: _hook
    mod.set_axon_ntff_profile_hook = lambda h: None
    sys.modules["antenv.axon_hooks"] = mod

    import concourse.bass_utils as bu

    bu.upload_artifacts = lambda tmpdir: str(tmpdir)
    _CACHE["hook_installed"] = True


def kernel(**inputs):
    if "nc" not in _CACHE:
        _CACHE["nc"] = _build_module()
    nc = _CACHE["nc"]

    in_maps = _host_prep(**inputs)
    trace = os.environ.get("KERNEL_TRACE", "0") == "1"
    tmpdir = None
    if trace:
        _install_ntff_hook()
        tmpdir = os.environ.get("KERNEL_TRACE_DIR") or None
    res = run_bass_kernel_spmd(nc, in_maps, core_ids=list(range(NCORES)), trace=trace,
                               tmpdir=tmpdir)
    if trace:
        _CACHE["last_exec_time_ns"] = res.exec_time_ns
        _CACHE["last_results"] = res

    y = np.concatenate([res.results[c]["y8"] for c in range(NCORES)], axis=0)
    return y



# revision 4
# speedup vs baseline: 1.3636x; 1.3636x over previous
"""Trainium2 Bass kernel for BEiT attention block (nn_Beit_9560597201107).

Data-parallel over batch: 64 batches -> 8 NeuronCores x 8 batches each.
Fully transposed dataflow (channels on partitions) so the softmax'd
attention matrix is never transposed on-chip:

  xT = x.T (host)                                  [768, 197]
  qkT[c, n] = sum_k WT[k, c] xT[k, n] + bias       [1536, 197]  (q pre-scaled)
  v[m, d]   = sum_k xT[k, m] WT_v[k, d] + bias     [197, 768]   (natural)
  scT[m, n] = sum_d kT[d, m] qT[d, n]              per head
  eT = exp(scT) * exp_rel_T                        (rel bias via exp-mult)
  po[d, n], sums[n] = sum_m [v|1][m, d] eT[m, n]   (ones col -> row 64 = sums)
  cT = po * broadcast(1/sums)   (PE ones-outer-product broadcast)
  y[n, o] = sum_c cT[c, n] projWT[c, o] + bias

All matmuls run in bfloat16 (fp32 PSUM accumulation): 1 cycle/row at any
free size (no 256-pad needed) and 2x faster FWL weight loads vs fp32r.
"""

import os
import numpy as np
import ml_dtypes

import concourse.bass as bass
import concourse.bacc as bacc
import concourse.mybir as mybir
import concourse.tile as tile
from concourse.bass_utils import run_bass_kernel_spmd
from concourse.bass_interp import get_hw_module

B, N, DIM, HEADS, NBS = 64, 197, 768, 12, 10
HEAD_DIM = DIM // HEADS
SCALE = HEAD_DIM ** -0.5
NCORES = 8
BPC = B // NCORES          # batches per core
KT = DIM // 128            # 6 contraction tiles
NF = N                     # token free-dim, exact (bf16 full rate at any size)
NF2 = 2 * NF
TOK_TILES = [(0, 128), (128, 69)]  # (offset, size) over the 197 tokens
# Heads grouped in same-parity pairs: both heads of a pair live at the same
# 64-partition half of qkT, so their back-to-back matmuls into one PSUM bank
# use the same PE row group.
PAIRS = [(0, 2), (4, 6), (8, 10), (1, 3), (5, 7), (9, 11)]
PAIR_PERM = [h for p in PAIRS for h in p]

F32 = mybir.dt.float32
BF16 = mybir.dt.bfloat16
BFNP = ml_dtypes.bfloat16

_CACHE = {}


def _build_module():
    nc = bacc.Bacc("TRN2", target_bir_lowering=False, debug=False)

    # host-transposed x: xt8[b, k, p, n] = x[b, n, 128k+p]
    xt8_d = nc.dram_tensor("xt8", [BPC, KT, 128, NF], BF16, kind="ExternalInput")
    wt_d = nc.dram_tensor("wt", [KT, 128, 3 * DIM], BF16, kind="ExternalInput")
    pwt_d = nc.dram_tensor("pwt", [KT, 128, DIM], BF16, kind="ExternalInput")
    qbc_d = nc.dram_tensor("qbc", [128, BPC, KT], F32, kind="ExternalInput")
    vpb_d = nc.dram_tensor("vpb8", [BPC, DIM], BF16, kind="ExternalInput")
    relt_d = nc.dram_tensor("relt", [6, 2, 128, NF2], BF16, kind="ExternalInput")
    ones_d = nc.dram_tensor("ones1", [1, NF], BF16, kind="ExternalInput")
    allones_d = nc.dram_tensor("allones", [128, 64], BF16, kind="ExternalInput")
    y8_d = nc.dram_tensor("y8", [BPC, N, DIM], F32, kind="ExternalOutput")

    with tile.TileContext(nc) as tc:
        with (
            tc.tile_pool(name="const", bufs=1) as constp,
            tc.tile_pool(name="sb_xT", bufs=2) as sb_xT,
            tc.tile_pool(name="sb_qkT", bufs=2) as sb_qkT,
            tc.tile_pool(name="sb_v", bufs=2) as sb_v,
            tc.tile_pool(name="sb_exp", bufs=2) as sb_exp,
            tc.tile_pool(name="sb_po", bufs=2) as sb_po,
            tc.tile_pool(name="sb_pohi", bufs=2) as sb_pohi,
            tc.tile_pool(name="sb_rec", bufs=2) as sb_rec,
            tc.tile_pool(name="sb_cT", bufs=2) as sb_cT,
            tc.tile_pool(name="sb_out", bufs=2) as sb_out,
            tc.tile_pool(name="sb_vpb", bufs=2) as sb_vpb,
            tc.tile_pool(name="ps", bufs=6, space="PSUM") as ps,
            tc.tile_pool(name="ps_pb", bufs=2, space="PSUM") as ps_pb,
        ):
            # ---- persistent data (loaded once) ----
            wt_sb = constp.tile([128, KT, 3 * DIM], BF16)
            nc.gpsimd.dma_start(out=wt_sb[:], in_=wt_d.ap().transpose([1, 0, 2]))
            pwt_sb = constp.tile([128, KT, DIM], BF16)
            nc.gpsimd.dma_start(out=pwt_sb[:], in_=pwt_d.ap().transpose([1, 0, 2]))
            relt_sb = constp.tile([128, 6, 2, NF2], BF16)
            nc.gpsimd.dma_start(out=relt_sb[:], in_=relt_d.ap().transpose([2, 0, 1, 3]))
            qbc_sb = constp.tile([128, BPC, KT], F32)
            nc.sync.dma_start(out=qbc_sb[:], in_=qbc_d.ap())

            ones_sb = constp.tile([1, NF], BF16)
            nc.gpsimd.dma_start(out=ones_sb[:], in_=ones_d.ap())
            allones_sb = constp.tile([128, 64], BF16)
            nc.gpsimd.dma_start(out=allones_sb[:], in_=allones_d.ap())

            def kT(qkT_sb, h, hb, off, mt):
                base = (h % 2) * 64
                return qkT_sb[base:base + 64, 6 + h // 2,
                              hb * NF + off:hb * NF + off + mt]

            def qT(qkT_sb, h, hb):
                base = (h % 2) * 64
                return qkT_sb[base:base + 64, h // 2, hb * NF:(hb + 1) * NF]

            prev_proj = [None]
            proj_state = {}

            def emit_proj_chunk(step, drain=False):
                if prev_proj[0] is None:
                    return
                pb_, cT_, vpb_ = prev_proj[0]
                if step == 0 and not drain:
                    proj_state.clear()
                chunks = [(0, 0), (0, 1), (0, 2), (1, 0), (1, 1), (1, 2)]
                todo = chunks if drain else [chunks[step]]
                for (t, jc) in todo:
                    off, mt = TOK_TILES[t]
                    if jc == 0:
                        proj_state[t] = (
                            ps.tile([128, 512], F32, tag="ps", name=f"pr_{pb_}_{t}"),
                            ps.tile([128, 256], F32, tag="ps", name=f"pr2_{pb_}_{t}"),
                        )
                    pr, pr2 = proj_state[t]
                    for j in (2 * jc, 2 * jc + 1):
                        nc.tensor.matmul(
                            pr[0:mt, :], cT_[:, j, off:off + mt], pwt_sb[:, j, 0:512],
                            start=(j == 0), stop=False,
                        )
                        nc.tensor.matmul(
                            pr2[0:mt, :], cT_[:, j, off:off + mt], pwt_sb[:, j, 512:768],
                            start=(j == 0), stop=False,
                        )
                    if jc == 2:
                        nc.tensor.matmul(
                            pr[0:mt, :], ones_sb[0:1, 0:mt], vpb_[0:1, 0:512],
                            start=False, stop=True,
                        )
                        nc.tensor.matmul(
                            pr2[0:mt, :], ones_sb[0:1, 0:mt], vpb_[0:1, 512:768],
                            start=False, stop=True,
                        )
                        out_sb = sb_out.tile([128, DIM], F32, tag="out",
                                             name=f"out_{pb_}_{t}")
                        nc.scalar.copy(out_sb[0:mt, 0:512], pr[0:mt, :])
                        nc.vector.tensor_copy(out_sb[0:mt, 512:768], pr2[0:mt, :])
                        nc.sync.dma_start(out=y8_d.ap()[pb_, off:off + mt, :],
                                          in_=out_sb[0:mt, :])
                if drain:
                    prev_proj[0] = None

            for g in range(BPC // 2):
                # ---- load host-transposed x for the batch pair ----
                xT_sb = sb_xT.tile([128, KT, NF2], BF16, tag="xT", name=f"xT_{g}")
                for hb in range(2):
                    nc.gpsimd.dma_start(
                        out=xT_sb[:, :, hb * NF:(hb + 1) * NF],
                        in_=xt8_d.ap()[2 * g + hb].transpose([1, 0, 2]),
                    )

                # ---- qkT for both batches (one weight load per block) ----
                qkT_sb = sb_qkT.tile([128, 12, NF2], BF16, tag="qkT", name=f"qkT_{g}")
                for ct in range(12):
                    qp = ps.tile([128, NF2], F32, tag="ps", name=f"qp_{g}_{ct}")
                    for k in range(KT):
                        nc.tensor.matmul(
                            qp[:],
                            wt_sb[:, k, ct * 128:(ct + 1) * 128],
                            xT_sb[:, k, :],
                            start=(k == 0),
                            stop=(k == KT - 1),
                        )
                    if ct < 6:
                        for hb in range(2):
                            qbias = qbc_sb[:, 2 * g + hb, ct:ct + 1]
                            dst = qkT_sb[:, ct, hb * NF:(hb + 1) * NF]
                            srcp = qp[:, hb * NF:(hb + 1) * NF]
                            if ct % 2 == 0:
                                nc.vector.tensor_scalar_add(dst, srcp, qbias)
                            else:
                                nc.scalar.activation(
                                    dst, srcp,
                                    mybir.ActivationFunctionType.Identity, bias=qbias,
                                )
                    else:
                        if ct % 2 == 0:
                            nc.vector.tensor_copy(qkT_sb[:, ct, :], qp[:])
                        else:
                            nc.scalar.copy(qkT_sb[:, ct, :], qp[:])

                for hb in range(2):
                    b = 2 * g + hb

                    vpb_t = sb_vpb.tile([1, DIM], BF16, tag="vpb", name=f"vpb_{b}")
                    nc.gpsimd.dma_start(out=vpb_t[:], in_=vpb_d.ap()[b].unsqueeze(0))

                    # ---- v (natural layout, 65-wide head slots, col 64 = 1s) ----
                    v_sb = sb_v.tile([128, 2, HEADS, 65], BF16, tag="v",
                                     name=f"v_{b}")
                    nc.gpsimd.memset(v_sb[:, :, :, 64:65], 1.0)
                    for t, (off, mt) in enumerate(TOK_TILES):
                        vp = ps.tile([128, 512], F32, tag="ps", name=f"vp_{b}_{t}")
                        vp2 = ps.tile([128, 256], F32, tag="ps", name=f"vp2_{b}_{t}")
                        for k in range(KT):
                            xsl = xT_sb[:, k, hb * NF + off:hb * NF + off + mt]
                            nc.tensor.matmul(
                                vp[0:mt, :], xsl, wt_sb[:, k, 1536:2048],
                                start=(k == 0), stop=(k == KT - 1),
                            )
                            nc.tensor.matmul(
                                vp2[0:mt, :], xsl, wt_sb[:, k, 2048:2304],
                                start=(k == 0), stop=(k == KT - 1),
                            )
                        # v_sb head axis is in PAIR_PERM order: even head h ->
                        # slot h//2, odd head h -> slot 6 + h//2
                        nc.vector.tensor_copy(
                            v_sb[0:mt, t, :, :].rearrange(
                                "p (par a) c -> p a par c", par=2)[:, 0:4, :, 0:64],
                            vp[0:mt, :].rearrange("p (a par d) -> p a par d",
                                                  par=2, d=HEAD_DIM),
                        )
                        nc.scalar.copy(
                            v_sb[0:mt, t, :, :].rearrange(
                                "p (par a) c -> p a par c", par=2)[:, 4:6, :, 0:64],
                            vp2[0:mt, :].rearrange("p (a par d) -> p a par d",
                                                   par=2, d=HEAD_DIM),
                        )

                    # ---- attention; prev-sp normalization and prev-batch ----
                    # ---- proj interleaved into each step                 ----
                    cT_sb = sb_cT.tile([128, 6, NF], BF16, tag="cT", name=f"cT_{b}")
                    norm_state = {}

                    def emit_norm(sp):
                        # pb broadcast + cT multiply for pair sp (deferred one
                        # step so the PE never waits on the reciprocal chain)
                        po_sb, po_hi, rec_sb = norm_state[sp]
                        pb = ps_pb.tile([128, NF], F32, tag="pb",
                                        name=f"pb_{b}_{sp}")
                        nc.tensor.matmul(
                            pb[0:64, :], allones_sb[64:65, 0:64],
                            rec_sb[64:65, 0:NF], start=True, stop=True,
                        )
                        nc.tensor.matmul(
                            pb[64:128, :], allones_sb[64:65, 0:64],
                            rec_sb[64:65, NF:NF2], start=True, stop=True,
                        )
                        nc.vector.tensor_mul(cT_sb[0:64, sp, :],
                                             po_sb[0:64, 0:NF], pb[0:64, :])
                        nc.vector.tensor_mul(cT_sb[64:128, sp, :],
                                             po_hi[64:128, :], pb[64:128, :])

                    for sp in range(6):
                        h0, h1 = PAIRS[sp]
                        expT = sb_exp.tile([128, 2, NF2], BF16, tag="expT",
                                           name=f"expT_{b}_{sp}")
                        scs = []
                        for t, (off, mt) in enumerate(TOK_TILES):
                            sc = ps.tile([128, NF2], F32, tag="ps",
                                         name=f"sc_{b}_{sp}_{t}")
                            nc.tensor.matmul(
                                sc[0:mt, 0:NF], kT(qkT_sb, h0, hb, off, mt),
                                qT(qkT_sb, h0, hb), start=True, stop=True,
                            )
                            nc.tensor.matmul(
                                sc[0:mt, NF:NF2], kT(qkT_sb, h1, hb, off, mt),
                                qT(qkT_sb, h1, hb), start=True, stop=True,
                            )
                            scs.append(sc)
                        for t, (off, mt) in enumerate(TOK_TILES):
                            nc.scalar.activation(
                                expT[0:mt, t, :], scs[t][0:mt, :],
                                mybir.ActivationFunctionType.Exp,
                            )
                            nc.vector.tensor_mul(
                                expT[0:mt, t, :], expT[0:mt, t, :],
                                relt_sb[0:mt, sp, t, :],
                            )
                        if sp > 0:
                            emit_norm(sp - 1)
                        # long warm proj matmuls of the previous batch fill the
                        # exp/mult wait and keep the PE clock-gate open
                        emit_proj_chunk(sp)
                        # one PSUM bank per head: a start=True resets the
                        # whole bank's has-written bits, so accumulation
                        # groups must not interleave within a bank
                        pos = [ps.tile([65, NF], F32, tag="ps",
                                       name=f"po_{b}_{sp}_{i}") for i in (0, 1)]
                        for i in (0, 1):
                            for t, (off, mt) in enumerate(TOK_TILES):
                                nc.tensor.matmul(
                                    pos[i][0:65, :],
                                    v_sb[0:mt, t, 2 * sp + i, :],
                                    expT[0:mt, t, i * NF:(i + 1) * NF],
                                    start=(t == 0), stop=(t == 1),
                                )
                        # row 64 of po = per-token exp sums; normalize later
                        po_sb = sb_po.tile([65, NF2], F32, tag="po",
                                           name=f"po_sb_{b}_{sp}")
                        nc.vector.tensor_copy(po_sb[0:65, 0:NF], pos[0][0:65, :])
                        nc.scalar.copy(po_sb[0:65, NF:NF2], pos[1][0:65, :])
                        rec_f = sb_rec.tile([65, NF2], F32, tag="recf",
                                            name=f"recf_{b}_{sp}")
                        # rows 0:64 are po values (unused garbage out); the
                        # custom DVE op needs base partition 0 to work
                        nc.vector.reciprocal_approx_fast(out=rec_f[0:65, :],
                                                         in_=po_sb[0:65, :])
                        rec_sb = sb_rec.tile([65, NF2], BF16, tag="rec",
                                             name=f"rec_{b}_{sp}")
                        nc.scalar.copy(rec_sb[64:65, :], rec_f[64:65, :])
                        # h1's po block must move to partitions 64:128 for cT
                        po_hi = sb_pohi.tile([128, NF], F32, tag="pohi",
                                             name=f"pohi_{b}_{sp}")
                        nc.sync.dma_start(out=po_hi[64:128, :],
                                          in_=po_sb[0:64, NF:NF2])
                        norm_state[sp] = (po_sb, po_hi, rec_sb)
                    emit_norm(5)
                    prev_proj[0] = (b, cT_sb, vpb_t)

            # drain the last batch's projection
            emit_proj_chunk(0, drain=True)

    nc.compile()
    nc.m = get_hw_module(nc.m)
    return nc


def _host_prep(x, qkv_weight, q_bias, v_bias, rel_table, proj_weight, proj_bias,
               b_idx, rel_index):
    x = np.asarray(x, dtype=np.float32)
    # xt8[b, k, p, n] = x[b, n, 128k+p]
    xt = np.ascontiguousarray(
        x.transpose(0, 2, 1).reshape(B, KT, 128, N)).astype(BFNP)
    W = np.asarray(qkv_weight, dtype=np.float32).copy()
    W[:DIM] *= np.float32(SCALE)
    wt = np.ascontiguousarray(W.T.reshape(KT, 128, 3 * DIM)).astype(BFNP)
    pwtT = np.asarray(proj_weight, dtype=np.float32).T  # [c', o]
    pwtT = pwtT.reshape(HEADS, HEAD_DIM, DIM)[PAIR_PERM].reshape(DIM, DIM)
    pwt = np.ascontiguousarray(pwtT.reshape(KT, 128, DIM)).astype(BFNP)

    bi = np.asarray(b_idx).astype(np.int64)
    qb_all = (np.asarray(q_bias, dtype=np.float32)[bi] * np.float32(SCALE))
    vb_all = np.asarray(v_bias, dtype=np.float32)[bi]
    # softmax rows sum to 1, so attn @ (1 x vb) == 1 x vb; push the v bias
    # through the projection into the proj bias
    pb_all = (np.asarray(proj_bias, dtype=np.float32)[bi]
              + vb_all @ np.asarray(proj_weight, dtype=np.float32).T).astype(BFNP)

    ridx = np.asarray(rel_index).astype(np.int64)
    rel = np.asarray(rel_table, dtype=np.float32)[ridx.reshape(-1)]
    rel = rel.reshape(N, N, HEADS)  # [n, m, h]
    relth = np.zeros((HEADS, 2, 128, NF), dtype=np.float32)
    for t, (off, mt) in enumerate(TOK_TILES):
        # relth[h, t, p, n] = exp(rel[n, off+p, h])
        relth[:, t, 0:mt, :] = np.exp(rel[:, off:off + mt, :].transpose(2, 1, 0))
    # pair-merged: relt[sp, t, p, i*NF+n] = relth[PAIRS[sp][i], t, p, n]
    relt = np.ascontiguousarray(
        relth[PAIR_PERM].reshape(6, 2, 2, 128, NF)
        .transpose(0, 2, 3, 1, 4).reshape(6, 2, 128, NF2)).astype(BFNP)

    ones1 = np.ones((1, NF), dtype=BFNP)
    allones = np.ones((128, 64), dtype=BFNP)

    in_maps = []
    for c in range(NCORES):
        sl = slice(c * BPC, (c + 1) * BPC)
        qbc = np.ascontiguousarray(
            qb_all[sl].reshape(BPC, KT, 128).transpose(2, 0, 1))
        vpb = np.ascontiguousarray(pb_all[sl])
        in_maps.append({
            "xt8": np.ascontiguousarray(xt[sl]),
            "wt": wt,
            "pwt": pwt,
            "qbc": qbc,
            "vpb8": vpb,
            "relt": relt,
            "ones1": ones1,
            "allones": allones,
        })
    return in_maps


def _install_ntff_hook():
    """Provide antenv.axon_hooks (absent from this image) so bass_utils can
    capture NTFF profiles through libaxon_pjrt.so, and keep artifacts local."""
    if _CACHE.get("hook_installed"):
        return
    import sys
    import types
    import ctypes
    import contextlib

    so_path = "/opt/axon/libaxon_pjrt.so"
    lib = ctypes.CDLL(so_path)
    lib.axon_start_nrt_profile.argtypes = [
        ctypes.POINTER(ctypes.c_int64),
        ctypes.c_size_t,
    ]
    lib.axon_start_nrt_profile.restype = ctypes.c_int64
    lib.axon_stop_nrt_profile.argtypes = [ctypes.c_char_p]
    lib.axon_stop_nrt_profile.restype = ctypes.c_int64

    @contextlib.contextmanager
    def _hook(output_dir, device_ids):
        import jax

        jax.devices()
        if device_ids:
            ids = (ctypes.c_int64 * len(device_ids))(*device_ids)
            rc = lib.axon_start_nrt_profile(ids, len(device_ids))
        else:
            rc = lib.axon_start_nrt_profile(None, 0)
        if rc != 0:
            raise RuntimeError(f"axon_start_nrt_profile rc={rc}")
        try:
            yield
        finally:
            n = lib.axon_stop_nrt_profile(str(output_dir).encode())
            print(f"ntff profile: {n} file(s) written to {output_dir}")

    mod = types.ModuleType("antenv.axon_hooks")
    mod.get_axon_ntff_profile_hook = lambda: _hook
    mod.set_axon_ntff_profile_hook = lambda h: None
    sys.modules["antenv.axon_hooks"] = mod

    import concourse.bass_utils as bu

    bu.upload_artifacts = lambda tmpdir: str(tmpdir)
    _CACHE["hook_installed"] = True


def kernel(**inputs):
    if "nc" not in _CACHE:
        _CACHE["nc"] = _build_module()
    nc = _CACHE["nc"]

    in_maps = _host_prep(**inputs)
    trace = os.environ.get("KERNEL_TRACE", "0") == "1"
    tmpdir = None
    if trace:
        _install_ntff_hook()
        tmpdir = os.environ.get("KERNEL_TRACE_DIR") or None
    res = run_bass_kernel_spmd(nc, in_maps, core_ids=list(range(NCORES)), trace=trace,
                               tmpdir=tmpdir)
    if trace:
        _CACHE["last_exec_time_ns"] = res.exec_time_ns
        _CACHE["last_results"] = res

    y = np.concatenate([res.results[c]["y8"] for c in range(NCORES)], axis=0)
    return y
